# revision 18
# baseline (speedup 1.0000x reference)
# Trainium2 Bass kernel for CrossAttentionPro:
#   q = x@Wq; k,v = context@Wkv; A = softmax(q k^T / sqrt(d));
#   A = depthwise3x3(A) + conv_b; out = (A @ v) merged @ Wp + bp
#
# Distribution: data-parallel over batch, one batch element per NeuronCore (B=8).
#
# Algorithm (per core, per head):
#   - Host pre-transposes x/context and pre-casts weights to 16-bit, so no
#     on-device transposes are needed.
#   - Keep scores transposed: S^T[m,n] tiles via matmul(lhsT=kT[d,m], rhs=qT[d,n])
#   - exp(scale*S^T) fused on ScalarE, PSUM->SBUF (fp8e4 when FP8_PV).
#   - The depthwise 3x3 conv is folded into the PV stationaries: during setup,
#     U_i(h) = sum_j w[i,j](h) * V_shift(1-j)(h) is built elementwise on
#     DVE/Pool (combos hidden under the q/k projections). The PV matmuls then
#     produce T_i = U_i^T expS^T directly, and the 9-tap combine collapses to
#     two shifted adds.  The softmax denominator comes for free as a 65th
#     "ones" column in the U_2 stationary group.
#   - With FP8_PV, U/expS are fp8e4 and the PV matmuls run in DoubleRow mode
#     (two m-tiles of K per matmul).
#   - T_i tiles are cast/normalized PSUM->SBUF f16 quickly (freeing PSUM for
#     the next head); denominator reciprocal uses the fast approx DVE op.
#   - Result tiles are out^T [C,N] f16 which feed the final projection directly
#     as matmul stationaries.

import os

import numpy as np

B, N, M, C, H = 8, 1024, 1024, 768, 12
D = C // H  # 64
NCORES = 8
FP8_PV = False


def _chunks(total, size):
    out = []
    s = 0
    while s < total:
        out.append((s, min(size, total - s)))
        s += size
    return out


def build_bass(cfg=None):
    """Builds the single-core Bass program (SPMD across cores via in_maps)."""
    import concourse.bass as bass
    import concourse.mybir as mybir
    import concourse.tile as tile
    from concourse import bacc

    cfg = cfg or {}
    n = cfg.get("N", N)
    m = cfg.get("M", M)
    c = cfg.get("C", C)
    h = cfg.get("H", H)
    fp8_pv = cfg.get("fp8_pv", FP8_PV)
    d = c // h
    assert d == 64 and h % 2 == 0 and n % 128 == 0 and m % 128 == 0 and c % 128 == 0

    fp32 = mybir.dt.float32
    bf16 = mybir.dt.bfloat16
    f16 = mybir.dt.float16
    f8 = mybir.dt.float8e4
    udt = f8 if fp8_pv else bf16
    F = mybir.ActivationFunctionType
    A = mybir.AluOpType
    PSUM = bass.MemorySpace.PSUM
    DR = mybir.MatmulPerfMode.DoubleRow

    KT = c // 128      # c tiles
    NT = n // 128      # n (query) tiles
    MT = m // 128      # m (key) tiles
    HP = h // 2        # head pairs
    scale = d ** -0.5

    nc = bacc.Bacc("TRN2", target_bir_lowering=False, debug=False,
                   num_devices=cfg.get("num_devices", NCORES))

    xT_d = nc.dram_tensor("xT", (c, n), bf16, kind="ExternalInput")
    cT_d = nc.dram_tensor("cT", (c, m), bf16, kind="ExternalInput")
    wq_d = nc.dram_tensor("wq", (c, c), bf16, kind="ExternalInput")
    wkv_d = nc.dram_tensor("wkv", (c, 2 * c), bf16, kind="ExternalInput")
    wp_d = nc.dram_tensor("wp", (c, c), f16, kind="ExternalInput")
    bp_d = nc.dram_tensor("bp", (1, c), f16, kind="ExternalInput")
    # wpat[ij*128 + p, col] = conv_w[col//64, 0, i, j]  (same for all p)
    wpat_d = nc.dram_tensor("wpat", (9 * 128, c), bf16, kind="ExternalInput")
    # bvec[p, hp] = conv_b[2*hp + p//64]
    bvec_d = nc.dram_tensor("bvec", (128, HP), fp32, kind="ExternalInput")
    out_d = nc.dram_tensor("out", (n, c), fp32, kind="ExternalOutput")

    with tile.TileContext(nc) as tc:
        with tc.tile_pool(name="const", bufs=1) as const, \
             tc.tile_pool(name="persist", bufs=1) as persist, \
             tc.tile_pool(name="ps_s", bufs=2, space=PSUM) as ps_s:

            bvec = const.tile([128, HP], fp32, name="bvec", tag="bvec")
            nc.sync.dma_start(bvec[:], bvec_d[:])
            bias_sb = const.tile([128, HP], fp32, name="bias_sb", tag="bias_sb")
            onescol = const.tile([128, 1], bf16, name="onescol", tag="onescol")
            nc.vector.memset(onescol[:], 1.0)
            ones16 = const.tile([1, 128], f16, name="ones16", tag="ones16")
            nc.vector.memset(ones16[:], 1.0)
            bp_sb = const.tile([1, c], f16, name="bp_sb", tag="bp_sb")
            nc.sync.dma_start(bp_sb[:], bp_d[:])
            eshift = const.tile([128, 1], fp32, name="eshift", tag="eshift")
            nc.vector.memset(eshift[:], -2.0)
            wpat = [const.tile([128, c], bf16, name=f"wpat{ij}", tag=f"wpat{ij}")
                    for ij in range(9)]
            for ij in range(9):
                nc.sync.dma_start(wpat[ij][:], wpat_d[ij * 128:(ij + 1) * 128, :])

            # persistent SBUF tensors
            qT = [persist.tile([128, n], bf16, name=f"qT{i}", tag=f"qT{i}") for i in range(KT)]
            kT = [persist.tile([128, m], bf16, name=f"kT{i}", tag=f"kT{i}") for i in range(KT)]
            V = [persist.tile([128, c], bf16, name=f"V{t}", tag=f"V{t}") for t in range(MT)]
            # PV stationaries (conv-folded):
            #   UA packs [U_0(h) | U_1(h)] per head (128 cols each head)
            #   UB packs [U_2(h) | ones]   per head (65 cols each head)
            if fp8_pv:
                # DoubleRow layout: for each (m-tile pair tt, head hh) the
                # 2x128 (resp. 2x65) weight planes must be contiguous so the
                # LDWEIGHTS access pattern collapses to 2D.
                UA = persist.tile([128, (MT // 2) * h * 2 * 128], f8,
                                  name="UA", tag="UA")
                # UB planes padded to 128 cols ([U_2 | ones | zeros]):
                # DoubleRow ldweights only encodes 32/64/128-wide planes.
                UB = persist.tile([128, (MT // 2) * h * 2 * 128], f8,
                                  name="UB", tag="UB")
                UA_r = UA.rearrange("p (tt hh two x) -> p tt hh two x",
                                    tt=MT // 2, hh=h, two=2)
                UB_r = UB.rearrange("p (tt hh two x) -> p tt hh two x",
                                    tt=MT // 2, hh=h, two=2)
                nc.gpsimd.memset(UB[:], 0.0)

                def uA_pair(tt, hh):
                    off = (tt * h + hh) * 2 * 128
                    return UA[:, off:off + 256].rearrange(
                        "p (two x) -> p two x", two=2)

                def uB_pair(tt, hh):
                    off = (tt * h + hh) * 2 * 128
                    return UB[:, off:off + 256].rearrange(
                        "p (two x) -> p two x", two=2)
            else:
                UAl = [persist.tile([128, 2 * c], bf16, name=f"UA{t}", tag=f"UA{t}")
                       for t in range(MT)]
                UBl = [persist.tile([128, 65 * h], bf16, name=f"UB{t}", tag=f"UB{t}")
                       for t in range(MT)]
            aT = [persist.tile([128, n], f16, name=f"aT{i}", tag=f"aT{i}") for i in range(HP)]
            wp_sb = [persist.tile([128, c], f16, name=f"wp{k}", tag=f"wp{k}") for k in range(KT)]

            # ---------------- phase 1+2: loads, projections, U combos -------
            with tc.tile_pool(name="ph1", bufs=1) as ph1, \
                 tc.tile_pool(name="vshift", bufs=3) as vshift, \
                 tc.tile_pool(name="combo", bufs=2) as combo, \
                 tc.tile_pool(name="dram", bufs=1, space=bass.MemorySpace.DRAM) as dram:

                xT = [ph1.tile([128, n], bf16, name=f"xT{i}", tag=f"xT{i}") for i in range(KT)]
                cT = [ph1.tile([128, m], bf16, name=f"cT{i}", tag=f"cT{i}") for i in range(KT)]
                wq_sb = [ph1.tile([128, c], bf16, name=f"wq{k}", tag=f"wq{k}") for k in range(KT)]
                wkv_sb = [ph1.tile([128, 2 * c], bf16, name=f"wkv{k}", tag=f"wkv{k}")
                          for k in range(KT)]

                for k in range(KT):
                    nc.sync.dma_start(cT[k][:], cT_d[k * 128:(k + 1) * 128, :])
                    nc.sync.dma_start(wkv_sb[k][:], wkv_d[k * 128:(k + 1) * 128, :])
                    nc.sync.dma_start(xT[k][:], xT_d[k * 128:(k + 1) * 128, :])
                    nc.sync.dma_start(wq_sb[k][:], wq_d[k * 128:(k + 1) * 128, :])
                    nc.sync.dma_start(wp_sb[k][:], wp_d[k * 128:(k + 1) * 128, :])

                # V first (the shifted-V roundtrip + U combos depend on it):
                # V[m-tile 128, c-chunk] = ctxT[k][:,m]^T . Wkv[k][:, c+cc]
                for t in range(MT):
                    pp = ps_s.tile([128, max(n, m, c)], fp32, name="ss", tag="ss")
                    for (c0, cl) in _chunks(c, 512):
                        for k in range(KT):
                            nc.tensor.matmul(
                                pp[:, c0:c0 + cl],
                                lhsT=cT[k][:, t * 128:(t + 1) * 128],
                                rhs=wkv_sb[k][:, c + c0:c + c0 + cl],
                                start=(k == 0), stop=(k == KT - 1))
                    nc.scalar.copy(V[t][:], pp[:, 0:c])

                # column sums of V per head pair -> conv bias vectors
                for hp in range(HP):
                    cs = ps_s.tile([128, max(n, m, c)], fp32, name="ss", tag="ss")
                    for t in range(MT):
                        nc.tensor.matmul(cs[:, 0:1],
                                         lhsT=V[t][:, hp * 128:(hp + 1) * 128],
                                         rhs=onescol[:], start=(t == 0),
                                         stop=(t == MT - 1))
                    nc.vector.tensor_tensor(bias_sb[:, hp:hp + 1], cs[:, 0:1],
                                            bvec[:, hp:hp + 1], op=A.mult)

                # +-1-row shifted V copies: shifts cross SBUF partition-tile
                # boundaries (start partitions must be 0/32/64/96), so
                # round-trip V through a zero-padded internal DRAM tensor.
                vdram = dram.tile([m + 2, c], bf16, name="vdram", tag="vdram")
                zrow = const.tile([1, c], bf16, name="zrow", tag="zrow")
                nc.vector.memset(zrow[:], 0.0)
                nc.sync.dma_start(vdram[0:1, :], zrow[:])
                nc.sync.dma_start(vdram[m + 1:m + 2, :], zrow[:])
                for t in range(MT):
                    nc.sync.dma_start(vdram[t * 128 + 1:(t + 1) * 128 + 1, :], V[t][:])

                # U combos per m-tile: U_i = w_i0*Vup + w_i1*V + w_i2*Vdn,
                # written straight into the packed UA/UB column layout.
                # Pool takes one mult per i, DVE the rest (Pool can't see PSUM
                # and has no TensorScalarPtr, but tensor_tensor is fine).
                for t in range(MT):
                    vup = vshift.tile([128, c], bf16, name="vup", tag="vup")
                    nc.sync.dma_start(vup[:], vdram[t * 128 + 2:t * 128 + 130, :])
                    vdn = vshift.tile([128, c], bf16, name="vdn", tag="vdn")
                    nc.sync.dma_start(vdn[:], vdram[t * 128:t * 128 + 128, :])
                    if fp8_pv:
                        rA = UA_r[:, t // 2, :, t % 2, :]
                        rB = UB_r[:, t // 2, :, t % 2, :]
                    else:
                        rA = UAl[t].rearrange("p (hh x) -> p hh x", x=128)
                        rB = UBl[t].rearrange("p (hh x) -> p hh x", x=65)
                    for i in range(3):
                        if i == 0:
                            dst = rA[:, :, 0:64]
                        elif i == 1:
                            dst = rA[:, :, 64:128]
                        else:
                            dst = rB[:, :, 0:64]
                        m1 = combo.tile([128, c], bf16, name="m1", tag="m1")
                        nc.vector.tensor_tensor(m1[:], vup[:], wpat[3 * i][:],
                                                op=A.mult)
                        m2 = combo.tile([128, c], bf16, name="m2", tag="m2")
                        nc.vector.tensor_tensor(m2[:], V[t][:], wpat[3 * i + 1][:],
                                                op=A.mult)
                        m3 = combo.tile([128, c], bf16, name="m3", tag="m3")
                        nc.gpsimd.tensor_tensor(m3[:], vdn[:], wpat[3 * i + 2][:],
                                                op=A.mult)
                        nc.vector.tensor_tensor(m1[:], m1[:], m2[:], op=A.add)
                        nc.vector.tensor_tensor(
                            dst, m1[:].rearrange("p (hh x) -> p hh x", x=64),
                            m3[:].rearrange("p (hh x) -> p hh x", x=64), op=A.add)
                    nc.vector.memset(rB[:, :, 64:65], 1.0)

                # qT / kT projections (overlap the combos above):
                # out[cout 128, n-chunk] = sum_k W[k][:,cout]^T . xT[k][:, n]
                for co in range(KT):
                    for proj_w, srcT, dstT, width in ((wq_sb, xT, qT, n),
                                                      (wkv_sb, cT, kT, m)):
                        pp = ps_s.tile([128, max(n, m, c)], fp32, name="ss", tag="ss")
                        for (n0, nl) in _chunks(width, 512):
                            for k in range(KT):
                                nc.tensor.matmul(
                                    pp[:, n0:n0 + nl],
                                    lhsT=proj_w[k][:, co * 128:(co + 1) * 128],
                                    rhs=srcT[k][:, n0:n0 + nl],
                                    start=(k == 0), stop=(k == KT - 1))
                        nc.scalar.copy(dstT[co][:], pp[:, 0:width])

            # ---------------- phase 3: per-head attention ----------------
            with tc.tile_pool(name="exps", bufs=(4 if fp8_pv else 3)) as exps_pool, \
                 tc.tile_pool(name="rpool", bufs=2) as rpool, \
                 tc.tile_pool(name="xpool", bufs=2) as xpool, \
                 tc.tile_pool(name="pspool", bufs=2) as pspool, \
                 tc.tile_pool(name="bcpool", bufs=1) as bcpool, \
                 tc.tile_pool(name="bc16pool", bufs=2) as bc16pool, \
                 tc.tile_pool(name="srpool", bufs=2) as srpool, \
                 tc.tile_pool(name="ps_pa", bufs=1, space=PSUM) as ps_pa, \
                 tc.tile_pool(name="ps_pb", bufs=1, space=PSUM) as ps_pb:

                def emit_scores(hp):
                    expS = []
                    # scores + exp for both heads (K=64 matmuls in base-
                    # partition row groups 0/64)
                    for hi in (0, 1):
                        es = exps_pool.tile([128, MT, n], udt, name="expS",
                                            tag="expS")
                        expS.append(es)
                        r0, r1 = hi * 64, (hi + 1) * 64
                        for t in range(MT):
                            ss = ps_s.tile([128, max(n, m, c)], fp32, name="ss",
                                           tag="ss")
                            for (n0, nl) in _chunks(n, 512):
                                nc.tensor.matmul(
                                    ss[:, n0:n0 + nl],
                                    lhsT=kT[hp][r0:r1, t * 128:(t + 1) * 128],
                                    rhs=qT[hp][r0:r1, n0:n0 + nl])
                            # TRN fp8e4 saturates at +-240 and exp(S) can
                            # reach ~410; shift by e^-2 (cancels exactly in
                            # the softmax normalization).
                            nc.scalar.activation(es[:, t, :], ss[:, 0:n], F.Exp,
                                                 scale=scale,
                                                 bias=(eshift[:] if fp8_pv else 0.0))
                    return expS

                def emit_pv(hp, expS):
                    R = [rpool.tile([128, n], f16, name=f"R{j}", tag=f"R{j}")
                         for j in range(3)]
                    for hi in (0, 1):
                        hh = 2 * hp + hi
                        es = expS[hi]
                        pa = ps_pa.tile([128, n], fp32, name="pa", tag="pa")
                        pb = ps_pb.tile([128 if fp8_pv else 65, n], fp32,
                                        name="pb", tag="pb")
                        if fp8_pv:
                            for tt in range(MT // 2):
                                for (n0, nl) in _chunks(n, 512):
                                    nc.tensor.matmul(
                                        pa[:, n0:n0 + nl],
                                        lhsT=uA_pair(tt, hh),
                                        rhs=es[:, 2 * tt:2 * tt + 2, n0:n0 + nl],
                                        start=(tt == 0), stop=(tt == MT // 2 - 1),
                                        perf_mode=DR)
                                for (n0, nl) in _chunks(n, 512):
                                    nc.tensor.matmul(
                                        pb[:, n0:n0 + nl],
                                        lhsT=uB_pair(tt, hh),
                                        rhs=es[:, 2 * tt:2 * tt + 2, n0:n0 + nl],
                                        start=(tt == 0), stop=(tt == MT // 2 - 1),
                                        perf_mode=DR)
                        else:
                            for t in range(MT):
                                for (n0, nl) in _chunks(n, 512):
                                    nc.tensor.matmul(
                                        pa[:, n0:n0 + nl],
                                        lhsT=UAl[t][:, 128 * hh:128 * (hh + 1)],
                                        rhs=es[:, t, n0:n0 + nl],
                                        start=(t == 0), stop=(t == MT - 1))
                                for (n0, nl) in _chunks(n, 512):
                                    nc.tensor.matmul(
                                        pb[:, n0:n0 + nl],
                                        lhsT=UBl[t][:, 65 * hh:65 * (hh + 1)],
                                        rhs=es[:, t, n0:n0 + nl],
                                        start=(t == 0), stop=(t == MT - 1))
                        # Cast pa to SBUF f16 right away: frees its PSUM banks
                        # for the next head without waiting the normalization
                        # chain.
                        paS = pspool.tile([128, n], f16, name="paS", tag="paS")
                        nc.vector.tensor_copy(paS[:], pa[:])
                        # softmax denominator: broadcast the sums row to all
                        # partitions via a K=1 ones outer-product on the PE,
                        # then fast approx reciprocal.
                        srow = srpool.tile([1, n], f16, name="srow", tag="srow")
                        nc.vector.tensor_copy(srow[:], pb[64:65, :])
                        sb_ps = ps_s.tile([128, max(n, m, c)], fp32, name="ss",
                                          tag="ss")
                        for (n0, nl) in _chunks(n, 512):
                            nc.tensor.matmul(sb_ps[:, n0:n0 + nl], lhsT=ones16[:],
                                             rhs=srow[:, n0:n0 + nl])
                        rbc = bcpool.tile([128, n], fp32, name="rbc", tag="rbc")
                        nc.vector.reciprocal_approx_fast(rbc[:], sb_ps[:, 0:n])
                        rbc16 = bc16pool.tile([128, n], f16, name="rbc16",
                                              tag="rbc16")
                        nc.vector.tensor_copy(rbc16[:], rbc[:])
                        # R_i pair tiles (rows hi*64..): T_i * (1/sums).
                        r0, r1 = hi * 64, (hi + 1) * 64
                        nc.vector.tensor_tensor(R[0][r0:r1, :], paS[0:64, :],
                                                rbc16[0:64, :], op=A.mult)
                        nc.vector.tensor_tensor(R[1][r0:r1, :], paS[64:128, :],
                                                rbc16[64:128, :], op=A.mult)
                        nc.vector.tensor_tensor(R[2][r0:r1, :], pb[0:64, :],
                                                rbc16[0:64, :], op=A.mult)

                    # conv row-combine: out[:,nn] = bias + R_0[:,nn-1] +
                    # R_1[:,nn] + R_2[:,nn+1]  (SAME-padded edges)
                    X = xpool.tile([128, n], f16, name="X", tag="X")
                    nc.vector.tensor_tensor(X[:, 1:n], R[0][:, 0:n - 1],
                                            R[1][:, 1:n], op=A.add)
                    nc.vector.tensor_copy(X[:, 0:1], R[1][:, 0:1])
                    nc.vector.scalar_tensor_tensor(
                        aT[hp][:, 0:n - 1], R[2][:, 1:n], bias_sb[:, hp:hp + 1],
                        X[:, 0:n - 1], op0=A.add, op1=A.add)
                    nc.vector.tensor_scalar(
                        aT[hp][:, n - 1:n], X[:, n - 1:n], bias_sb[:, hp:hp + 1],
                        None, op0=A.add)

                prev = None
                for hp in range(HP):
                    cur = emit_scores(hp)
                    if prev is not None:
                        emit_pv(hp - 1, prev)
                    prev = cur
                emit_pv(HP - 1, prev)

            # ---------------- phase 4: output projection ----------------
            with tc.tile_pool(name="outpool", bufs=3) as outpool, \
                 tc.tile_pool(name="ps_f", bufs=2, space=PSUM) as ps_f:
                for t in range(NT):
                    pf = ps_f.tile([128, c], fp32, name="pf", tag="pf")
                    for (c0, cl) in _chunks(c, 512):
                        for k in range(KT):
                            nc.tensor.matmul(pf[:, c0:c0 + cl],
                                             lhsT=aT[k][:, t * 128:(t + 1) * 128],
                                             rhs=wp_sb[k][:, c0:c0 + cl],
                                             start=(k == 0), stop=False)
                        nc.tensor.matmul(pf[:, c0:c0 + cl], lhsT=ones16[:],
                                         rhs=bp_sb[:, c0:c0 + cl], start=False,
                                         stop=True)
                    ot = outpool.tile([128, c], fp32, name="ot", tag="ot")
                    nc.vector.tensor_copy(ot[:], pf[:])
                    nc.sync.dma_start(out_d[t * 128:(t + 1) * 128, :], ot[:])

    nc.compile()
    return nc


def make_host_inputs(x, context, Wq, Wkv, conv_w, conv_b, Wp, bp, cfg=None):
    import ml_dtypes

    cfg = cfg or {}
    h = cfg.get("H", H)
    c = cfg.get("C", C)
    d = c // h
    HP = h // 2
    bvec = np.empty((128, HP), np.float32)
    for hp in range(HP):
        for p in range(128):
            bvec[p, hp] = conv_b[2 * hp + p // 64]
    # wpat[ij] is the conv weight w[i,j] of each column's head, replicated
    # over all 128 partitions.
    wpat = np.empty((9, 128, c), np.float32)
    heads_of_col = np.arange(c) // d
    for i in range(3):
        for j in range(3):
            wpat[3 * i + j] = conv_w[heads_of_col, 0, i, j][None, :]
    shared = {
        "wq": np.ascontiguousarray(Wq).astype(ml_dtypes.bfloat16),
        "wkv": np.ascontiguousarray(Wkv).astype(ml_dtypes.bfloat16),
        "wp": np.ascontiguousarray(Wp).astype(np.float16),
        "bp": np.ascontiguousarray(bp).astype(np.float16).reshape(1, -1),
        "wpat": wpat.reshape(9 * 128, c).astype(ml_dtypes.bfloat16),
        "bvec": bvec,
    }
    in_maps = []
    for b in range(x.shape[0]):
        im = dict(shared)
        im["xT"] = np.ascontiguousarray(x[b].T).astype(ml_dtypes.bfloat16)
        im["cT"] = np.ascontiguousarray(context[b].T).astype(ml_dtypes.bfloat16)
        in_maps.append(im)
    return in_maps


def kernel(x, context, Wq, Wkv, conv_w, conv_b, Wp, bp):
    from concourse.bass_utils import run_bass_kernel_spmd

    x = np.asarray(x, np.float32)
    context = np.asarray(context, np.float32)
    Wq = np.asarray(Wq, np.float32)
    Wkv = np.asarray(Wkv, np.float32)
    conv_w = np.asarray(conv_w, np.float32)
    conv_b = np.asarray(conv_b, np.float32)
    Wp = np.asarray(Wp, np.float32)
    bp = np.asarray(bp, np.float32)

    nc = build_bass()
    in_maps = make_host_inputs(x, context, Wq, Wkv, conv_w, conv_b, Wp, bp)
    res = run_bass_kernel_spmd(nc, in_maps, core_ids=list(range(NCORES)),
                               trace=bool(int(os.environ.get("KERNEL_TRACE", "0"))))
    out = np.stack([r["out"] for r in res.results], axis=0)
    if res.exec_time_ns is not None:
        print(f"HW exec time: {res.exec_time_ns} ns")
    kernel.last_result = res
    return out


# revision 23
# speedup vs baseline: 1.0765x; 1.0765x over previous
# Trainium2 Bass kernel for CrossAttentionPro:
#   q = x@Wq; k,v = context@Wkv; A = softmax(q k^T / sqrt(d));
#   A = depthwise3x3(A) + conv_b; out = (A @ v) merged @ Wp + bp
#
# Distribution: data-parallel over batch, one batch element per NeuronCore (B=8).
#
# Algorithm (per core, per head):
#   - Host pre-transposes x/context and pre-casts weights to 16-bit; no
#     on-device transposes.
#   - Keep scores transposed: S^T[m,n] tiles via matmul(lhsT=kT[d,m], rhs=qT[d,n])
#   - exp(scale*S^T - 2.5) fused on ScalarE -> fp8e4 (the -2.5 shift keeps exp
#     under the TRN fp8e4 +-240 range and cancels in the normalization).
#   - Depthwise conv decomposes into 3 column-shifted V copies (m-shifted V
#     stationaries, fp8 DoubleRow-paired) and 3 row shifts (free-dim shifts of
#     the small P'^T = V_j^T @ expS^T results). Softmax denominator comes for
#     free as a "ones" column in the V_dn stationary group (padded to a
#     128-wide DoubleRow plane).
#   - PV matmuls run fp8 DoubleRow: two m-tiles of K per matmul, operand
#     layouts arranged so every access pattern collapses to 2D.
#   - The denominator reciprocal runs on the approx DVE op on the [1,n] row,
#     then GpSimd partition_broadcast replicates it (no PE/PSUM involved).
#   - 9-tap combine: two accumulators, DVE takes i=0/i=1 taps as fused
#     scalar_tensor_tensor, Pool takes the i=2 shifted adds on pre-scaled
#     tiles plus the final merge.
#   - Result tiles are out^T [C,N] f16 which feed the final projection
#     directly as matmul stationaries.

import os

import numpy as np

B, N, M, C, H = 8, 1024, 1024, 768, 12
D = C // H  # 64
NCORES = 8


def _chunks(total, size):
    out = []
    s = 0
    while s < total:
        out.append((s, min(size, total - s)))
        s += size
    return out


def build_bass(cfg=None):
    """Builds the single-core Bass program (SPMD across cores via in_maps)."""
    import concourse.bass as bass
    import concourse.mybir as mybir
    import concourse.tile as tile
    from concourse import bacc

    cfg = cfg or {}
    n = cfg.get("N", N)
    m = cfg.get("M", M)
    c = cfg.get("C", C)
    h = cfg.get("H", H)
    dr = cfg.get("dr", True)          # fp8 DoubleRow PV
    d = c // h
    assert d == 64 and h % 2 == 0 and n % 512 == 0 and m % 256 == 0 and c % 128 == 0

    fp32 = mybir.dt.float32
    bf16 = mybir.dt.bfloat16
    f16 = mybir.dt.float16
    f8 = mybir.dt.float8e4
    vdt = f8 if dr else bf16
    F = mybir.ActivationFunctionType
    A = mybir.AluOpType
    PSUM = bass.MemorySpace.PSUM
    DR = mybir.MatmulPerfMode.DoubleRow

    KT = c // 128      # c tiles
    NT = n // 128      # n (query) tiles
    MT = m // 128      # m (key) tiles
    NC = n // 512      # n chunks (psum bank width)
    HP = h // 2        # head pairs
    scale = d ** -0.5

    nc = bacc.Bacc("TRN2", target_bir_lowering=False, debug=False,
                   num_devices=cfg.get("num_devices", NCORES))

    xT_d = nc.dram_tensor("xT", (c, n), bf16, kind="ExternalInput")
    cT_d = nc.dram_tensor("cT", (c, m), bf16, kind="ExternalInput")
    wq_d = nc.dram_tensor("wq", (c, c), bf16, kind="ExternalInput")
    wkv_d = nc.dram_tensor("wkv", (c, 2 * c), bf16, kind="ExternalInput")
    wp_d = nc.dram_tensor("wp", (c, c), f16, kind="ExternalInput")
    bp_d = nc.dram_tensor("bp", (1, c), f16, kind="ExternalInput")
    # wtap[p, hp*9 + 3*i + j] = conv_w[2*hp + p//64, 0, i, j]
    wtap_d = nc.dram_tensor("wtap", (128, 9 * HP), fp32, kind="ExternalInput")
    # bvec[p, hp] = conv_b[2*hp + p//64]
    bvec_d = nc.dram_tensor("bvec", (128, HP), fp32, kind="ExternalInput")
    out_d = nc.dram_tensor("out", (n, c), fp32, kind="ExternalOutput")

    with tile.TileContext(nc) as tc:
        with tc.tile_pool(name="const", bufs=1) as const, \
             tc.tile_pool(name="persist", bufs=1) as persist, \
             tc.tile_pool(name="ps_s", bufs=2, space=PSUM) as ps_s:

            wtap = const.tile([128, 9 * HP], fp32, name="wtap", tag="wtap")
            nc.sync.dma_start(wtap[:], wtap_d[:])
            bvec = const.tile([128, HP], fp32, name="bvec", tag="bvec")
            nc.sync.dma_start(bvec[:], bvec_d[:])
            bias_sb = const.tile([128, HP], fp32, name="bias_sb", tag="bias_sb")
            onescol = const.tile([128, 1], bf16, name="onescol", tag="onescol")
            nc.vector.memset(onescol[:], 1.0)
            ones16 = const.tile([1, 128], f16, name="ones16", tag="ones16")
            nc.vector.memset(ones16[:], 1.0)
            bp_sb = const.tile([1, c], f16, name="bp_sb", tag="bp_sb")
            nc.sync.dma_start(bp_sb[:], bp_d[:])
            eshift = const.tile([128, 1], fp32, name="eshift", tag="eshift")
            nc.vector.memset(eshift[:], -2.5 if dr else 0.0)

            # persistent SBUF tensors
            qT = [persist.tile([128, n], bf16, name=f"qT{i}", tag=f"qT{i}") for i in range(KT)]
            kT = [persist.tile([128, m], bf16, name=f"kT{i}", tag=f"kT{i}") for i in range(KT)]
            V = [persist.tile([128, c], vdt, name=f"V{t}", tag=f"V{t}") for t in range(MT)]
            V16 = [persist.tile([128, c], bf16, name=f"V16_{t}", tag=f"V16_{t}")
                   for t in range(MT)] if dr else V
            # PV stationaries: VA packs [V_up | V_center] per head (128 cols),
            # VB packs [V_dn | ones | zero-pad] per head (128 cols; DoubleRow
            # ldweights only encodes 32/64/128-wide planes).  In DR mode the
            # two planes of an m-tile pair are contiguous per (pair, head).
            if dr:
                VA = persist.tile([128, (MT // 2) * h * 2 * 128], f8,
                                  name="VA", tag="VA")
                VB = persist.tile([128, (MT // 2) * h * 2 * 128], f8,
                                  name="VB", tag="VB")
                VA_r = VA.rearrange("p (tt hh two x) -> p tt hh two x",
                                    tt=MT // 2, hh=h, two=2)
                VB_r = VB.rearrange("p (tt hh two x) -> p tt hh two x",
                                    tt=MT // 2, hh=h, two=2)
                nc.gpsimd.memset(VB[:], 0.0)

                def vA_pair(tt, hh):
                    off = (tt * h + hh) * 256
                    return VA[:, off:off + 256].rearrange(
                        "p (two x) -> p two x", two=2)

                def vB_pair(tt, hh):
                    off = (tt * h + hh) * 256
                    return VB[:, off:off + 256].rearrange(
                        "p (two x) -> p two x", two=2)

                def rA(t):
                    return VA_r[:, t // 2, :, t % 2, :]

                def rB(t):
                    return VB_r[:, t // 2, :, t % 2, :]
            else:
                VAl = [persist.tile([128, 2 * c], bf16, name=f"VA{t}",
                                    tag=f"VA{t}") for t in range(MT)]
                VBl = [persist.tile([128, 65 * h], bf16, name=f"VB{t}",
                                    tag=f"VB{t}") for t in range(MT)]

                def rA(t):
                    return VAl[t].rearrange("p (hh x) -> p hh x", x=128)

                def rB(t):
                    return VBl[t].rearrange("p (hh x) -> p hh x", x=65)
            aT = [persist.tile([128, n], f16, name=f"aT{i}", tag=f"aT{i}") for i in range(HP)]
            wp_sb = [persist.tile([128, c], f16, name=f"wp{k}", tag=f"wp{k}") for k in range(KT)]

            # ---------------- phase 1+2: loads, projections, shifted V ------
            with tc.tile_pool(name="ph1", bufs=1) as ph1, \
                 tc.tile_pool(name="dram", bufs=1, space=bass.MemorySpace.DRAM) as dram:

                xT = [ph1.tile([128, n], bf16, name=f"xT{i}", tag=f"xT{i}") for i in range(KT)]
                cT = [ph1.tile([128, m], bf16, name=f"cT{i}", tag=f"cT{i}") for i in range(KT)]
                wq_sb = [ph1.tile([128, c], bf16, name=f"wq{k}", tag=f"wq{k}") for k in range(KT)]
                wkv_sb = [ph1.tile([128, 2 * c], bf16, name=f"wkv{k}", tag=f"wkv{k}")
                          for k in range(KT)]

                for k in range(KT):
                    nc.sync.dma_start(cT[k][:], cT_d[k * 128:(k + 1) * 128, :])
                    nc.sync.dma_start(wkv_sb[k][:], wkv_d[k * 128:(k + 1) * 128, :])
                    nc.sync.dma_start(xT[k][:], xT_d[k * 128:(k + 1) * 128, :])
                    nc.sync.dma_start(wq_sb[k][:], wq_d[k * 128:(k + 1) * 128, :])
                    nc.sync.dma_start(wp_sb[k][:], wp_d[k * 128:(k + 1) * 128, :])

                # V first (the shifted-V roundtrip depends on it):
                # V[m-tile 128, c-chunk] = ctxT[k][:,m]^T . Wkv[k][:, c+cc]
                for t in range(MT):
                    pp = ps_s.tile([128, max(n, m, c)], fp32, name="ss", tag="ss")
                    for (c0, cl) in _chunks(c, 512):
                        for k in range(KT):
                            nc.tensor.matmul(
                                pp[:, c0:c0 + cl],
                                lhsT=cT[k][:, t * 128:(t + 1) * 128],
                                rhs=wkv_sb[k][:, c + c0:c + c0 + cl],
                                start=(k == 0), stop=(k == KT - 1))
                    nc.scalar.copy(V[t][:], pp[:, 0:c])
                    if dr:
                        nc.scalar.copy(V16[t][:], pp[:, 0:c])

                # column sums of V per head pair -> conv bias vectors
                for hp in range(HP):
                    cs = ps_s.tile([128, max(n, m, c)], fp32, name="ss", tag="ss")
                    for t in range(MT):
                        nc.tensor.matmul(cs[:, 0:1],
                                         lhsT=V16[t][:, hp * 128:(hp + 1) * 128],
                                         rhs=onescol[:], start=(t == 0),
                                         stop=(t == MT - 1))
                    nc.vector.tensor_tensor(bias_sb[:, hp:hp + 1], cs[:, 0:1],
                                            bvec[:, hp:hp + 1], op=A.mult)

                # +-1-row shifted V stripes: shifts cross SBUF partition-tile
                # boundaries (engine/DMA start partitions must be 0/32/64/96),
                # so round-trip V through a zero-padded internal DRAM tensor.
                vdram = dram.tile([m + 2, c], vdt, name="vdram", tag="vdram")
                zrow = const.tile([1, c], vdt, name="zrow", tag="zrow")
                nc.vector.memset(zrow[:], 0.0)
                nc.sync.dma_start(vdram[0:1, :], zrow[:])
                nc.sync.dma_start(vdram[m + 1:m + 2, :], zrow[:])
                for t in range(MT):
                    nc.sync.dma_start(vdram[t * 128 + 1:(t + 1) * 128 + 1, :], V[t][:])
                for t in range(MT):
                    # center stripes straight from SBUF V
                    nc.sync.dma_start(rA(t)[:, :, 64:128],
                                      V[t].rearrange("p (hh x) -> p hh x", x=64))
                    # v[m = 128t + p + 1]: vdram rows [128t+2 : 128t+130]
                    nc.sync.dma_start(
                        rA(t)[:, :, 0:64],
                        vdram[t * 128 + 2:t * 128 + 130, :]
                        .rearrange("p (hh x) -> p hh x", x=64))
                    # v[m = 128t + p - 1]: vdram rows [128t : 128t+128]
                    nc.sync.dma_start(
                        rB(t)[:, :, 0:64],
                        vdram[t * 128:t * 128 + 128, :]
                        .rearrange("p (hh x) -> p hh x", x=64))
                    nc.vector.memset(rB(t)[:, :, 64:65], 1.0)

                # qT / kT projections; co order matches head pairs so scores
                # can start as soon as the first pair's tiles land.
                for co in range(KT):
                    for proj_w, srcT, dstT, width in ((wq_sb, xT, qT, n),
                                                      (wkv_sb, cT, kT, m)):
                        pp = ps_s.tile([128, max(n, m, c)], fp32, name="ss", tag="ss")
                        for (n0, nl) in _chunks(width, 512):
                            for k in range(KT):
                                nc.tensor.matmul(
                                    pp[:, n0:n0 + nl],
                                    lhsT=proj_w[k][:, co * 128:(co + 1) * 128],
                                    rhs=srcT[k][:, n0:n0 + nl],
                                    start=(k == 0), stop=(k == KT - 1))
                        nc.scalar.copy(dstT[co][:], pp[:, 0:width])

            # ---------------- phase 3: per-head attention ----------------
            with tc.tile_pool(name="exps", bufs=(4 if dr else 3)) as exps_pool, \
                 tc.tile_pool(name="qpool", bufs=2) as qpool, \
                 tc.tile_pool(name="accpool", bufs=2) as accpool, \
                 tc.tile_pool(name="pspool", bufs=2) as pspool, \
                 tc.tile_pool(name="wqpool", bufs=1) as wqpool, \
                 tc.tile_pool(name="srpool", bufs=2) as srpool, \
                 tc.tile_pool(name="bcpool", bufs=2) as bcpool, \
                 tc.tile_pool(name="ps_pa", bufs=1, space=PSUM) as ps_pa, \
                 tc.tile_pool(name="ps_pb", bufs=1, space=PSUM) as ps_pb:

                def emit_scores(hp):
                    expS = []
                    # scores + exp for both heads (K=64 matmuls in base-
                    # partition row groups 0/64). es layout in DR mode:
                    # [p, pair, chunk, plane, 512] so the DoubleRow rhs
                    # access pattern merges to 2D.
                    for hi in (0, 1):
                        es = exps_pool.tile(
                            [128, MT // 2, NC, 2, 512] if dr else [128, MT, n],
                            f8 if dr else bf16, name="expS", tag="expS")
                        expS.append(es)
                        r0, r1 = hi * 64, (hi + 1) * 64
                        for t in range(MT):
                            ss = ps_s.tile([128, max(n, m, c)], fp32,
                                           name="ss", tag="ss")
                            for (n0, nl) in _chunks(n, 512):
                                nc.tensor.matmul(
                                    ss[:, n0:n0 + nl],
                                    lhsT=kT[hp][r0:r1, t * 128:(t + 1) * 128],
                                    rhs=qT[hp][r0:r1, n0:n0 + nl])
                            dst = (es[:, t // 2, :, t % 2, :] if dr
                                   else es[:, t, :])
                            nc.scalar.activation(dst, ss[:, 0:n], F.Exp,
                                                 scale=scale, bias=eshift[:])
                    return expS

                def emit_pv(hp, expS):
                    Q = [qpool.tile([128, n], f16, name=f"Q{j}", tag=f"Q{j}")
                         for j in range(3)]
                    for hi in (0, 1):
                        hh = 2 * hp + hi
                        es = expS[hi]
                        pa = ps_pa.tile([128, n], fp32, name="pa", tag="pa")
                        pb = ps_pb.tile([128 if dr else 65, n], fp32,
                                        name="pb", tag="pb")
                        if dr:
                            for tt in range(MT // 2):
                                for ch in range(NC):
                                    nc.tensor.matmul(
                                        pa[:, ch * 512:(ch + 1) * 512],
                                        lhsT=vA_pair(tt, hh),
                                        rhs=es[:, tt, ch, :, :],
                                        start=(tt == 0),
                                        stop=(tt == MT // 2 - 1),
                                        perf_mode=DR)
                                for ch in range(NC):
                                    nc.tensor.matmul(
                                        pb[:, ch * 512:(ch + 1) * 512],
                                        lhsT=vB_pair(tt, hh),
                                        rhs=es[:, tt, ch, :, :],
                                        start=(tt == 0),
                                        stop=(tt == MT // 2 - 1),
                                        perf_mode=DR)
                        else:
                            for t in range(MT):
                                for (n0, nl) in _chunks(n, 512):
                                    nc.tensor.matmul(
                                        pa[:, n0:n0 + nl],
                                        lhsT=VAl[t][:, 128 * hh:128 * (hh + 1)],
                                        rhs=es[:, t, n0:n0 + nl],
                                        start=(t == 0), stop=(t == MT - 1))
                                for (n0, nl) in _chunks(n, 512):
                                    nc.tensor.matmul(
                                        pb[:, n0:n0 + nl],
                                        lhsT=VBl[t][:, 65 * hh:65 * (hh + 1)],
                                        rhs=es[:, t, n0:n0 + nl],
                                        start=(t == 0), stop=(t == MT - 1))
                        # cast pa to SBUF f16 right away: frees its PSUM for
                        # the next head without waiting the normalize chain
                        paS = pspool.tile([128, n], f16, name="paS", tag="paS")
                        nc.vector.tensor_copy(paS[:], pa[:])
                        # denominator row -> fast reciprocal -> f16 row ->
                        # partition broadcast (GpSimd; no PE/PSUM involved)
                        srow = srpool.tile([1, n], fp32, name="srow", tag="srow")
                        nc.scalar.copy(srow[:], pb[64:65, :])
                        rrow = srpool.tile([1, n], fp32, name="rrow", tag="rrow")
                        nc.vector.reciprocal_approx_fast(rrow[:], srow[:])
                        r16 = srpool.tile([1, n], f16, name="r16", tag="r16")
                        nc.vector.tensor_copy(r16[:], rrow[:])
                        rbc16 = bcpool.tile([128, n], f16, name="rbc16",
                                            tag="rbc16")
                        nc.gpsimd.partition_broadcast(rbc16[:], r16[:])
                        # Q_j pair tiles (rows hi*64..): P'_j * (1/sums)
                        r0, r1 = hi * 64, (hi + 1) * 64
                        nc.vector.tensor_tensor(Q[0][r0:r1, :], paS[0:64, :],
                                                rbc16[0:64, :], op=A.mult)
                        nc.vector.tensor_tensor(Q[1][r0:r1, :], paS[64:128, :],
                                                rbc16[64:128, :], op=A.mult)
                        nc.vector.tensor_tensor(Q[2][r0:r1, :], pb[0:64, :],
                                                rbc16[0:64, :], op=A.mult)

                    # 9-tap combine: out^T[p,nn] = bias + sum_ij w[i,j]*Q_j[p,nn+i-1]
                    # accA on DVE takes i=1 (+bias) and the fused i=0 taps;
                    # accB gets the i=2 taps as Pool shifted adds on DVE-
                    # prescaled tiles; Pool also does the final merge.
                    def wv(i, j):
                        return wtap[:, hp * 9 + 3 * i + j: hp * 9 + 3 * i + j + 1]

                    accA = accpool.tile([128, n], f16, name="accA", tag="accA")
                    accB = accpool.tile([128, n], f16, name="accB", tag="accB")
                    nc.vector.tensor_scalar(accA[:], Q[0][:], wv(1, 0),
                                            bias_sb[:, hp:hp + 1],
                                            op0=A.mult, op1=A.add)
                    nc.vector.scalar_tensor_tensor(accA[:], Q[1][:], wv(1, 1),
                                                   accA[:], op0=A.mult, op1=A.add)
                    for j in range(3):
                        nc.vector.scalar_tensor_tensor(
                            accA[:, 1:n], Q[j][:, 0:n - 1], wv(0, j),
                            accA[:, 1:n], op0=A.mult, op1=A.add)
                    nc.vector.tensor_scalar(accB[:], Q[2][:], wv(1, 2), None,
                                            op0=A.mult)
                    for j in range(3):
                        wQ = wqpool.tile([128, n], f16, name="wQ", tag=f"wQ{j}")
                        nc.vector.tensor_scalar(wQ[:], Q[j][:], wv(2, j), None,
                                                op0=A.mult)
                        nc.gpsimd.tensor_tensor(accB[:, 0:n - 1], wQ[:, 1:n],
                                                accB[:, 0:n - 1], op=A.add)
                    nc.gpsimd.tensor_tensor(aT[hp][:], accA[:], accB[:], op=A.add)

                prev = None
                for hp in range(HP):
                    cur = emit_scores(hp)
                    if prev is not None:
                        emit_pv(hp - 1, prev)
                    prev = cur
                emit_pv(HP - 1, prev)

            # ---------------- phase 4: output projection ----------------
            with tc.tile_pool(name="outpool", bufs=3) as outpool, \
                 tc.tile_pool(name="ps_f", bufs=2, space=PSUM) as ps_f:
                for t in range(NT):
                    pf = ps_f.tile([128, c], fp32, name="pf", tag="pf")
                    for (c0, cl) in _chunks(c, 512):
                        for k in range(KT):
                            nc.tensor.matmul(pf[:, c0:c0 + cl],
                                             lhsT=aT[k][:, t * 128:(t + 1) * 128],
                                             rhs=wp_sb[k][:, c0:c0 + cl],
                                             start=(k == 0), stop=False)
                        nc.tensor.matmul(pf[:, c0:c0 + cl], lhsT=ones16[:],
                                         rhs=bp_sb[:, c0:c0 + cl], start=False,
                                         stop=True)
                    ot = outpool.tile([128, c], fp32, name="ot", tag="ot")
                    nc.vector.tensor_copy(ot[:], pf[:])
                    nc.sync.dma_start(out_d[t * 128:(t + 1) * 128, :], ot[:])

    nc.compile()
    return nc


def make_host_inputs(x, context, Wq, Wkv, conv_w, conv_b, Wp, bp, cfg=None):
    import ml_dtypes

    cfg = cfg or {}
    h = cfg.get("H", H)
    HP = h // 2
    wtap = np.empty((128, 9 * HP), np.float32)
    bvec = np.empty((128, HP), np.float32)
    for hp in range(HP):
        for p in range(128):
            head = 2 * hp + p // 64
            bvec[p, hp] = conv_b[head]
            for i in range(3):
                for j in range(3):
                    wtap[p, hp * 9 + 3 * i + j] = conv_w[head, 0, i, j]
    shared = {
        "wq": np.ascontiguousarray(Wq).astype(ml_dtypes.bfloat16),
        "wkv": np.ascontiguousarray(Wkv).astype(ml_dtypes.bfloat16),
        "wp": np.ascontiguousarray(Wp).astype(np.float16),
        "bp": np.ascontiguousarray(bp).astype(np.float16).reshape(1, -1),
        "wtap": wtap,
        "bvec": bvec,
    }
    in_maps = []
    for b in range(x.shape[0]):
        im = dict(shared)
        im["xT"] = np.ascontiguousarray(x[b].T).astype(ml_dtypes.bfloat16)
        im["cT"] = np.ascontiguousarray(context[b].T).astype(ml_dtypes.bfloat16)
        in_maps.append(im)
    return in_maps


def kernel(x, context, Wq, Wkv, conv_w, conv_b, Wp, bp):
    from concourse.bass_utils import run_bass_kernel_spmd

    x = np.asarray(x, np.float32)
    context = np.asarray(context, np.float32)
    Wq = np.asarray(Wq, np.float32)
    Wkv = np.asarray(Wkv, np.float32)
    conv_w = np.asarray(conv_w, np.float32)
    conv_b = np.asarray(conv_b, np.float32)
    Wp = np.asarray(Wp, np.float32)
    bp = np.asarray(bp, np.float32)

    nc = build_bass()
    in_maps = make_host_inputs(x, context, Wq, Wkv, conv_w, conv_b, Wp, bp)
    res = run_bass_kernel_spmd(nc, in_maps, core_ids=list(range(NCORES)),
                               trace=bool(int(os.environ.get("KERNEL_TRACE", "0"))))
    out = np.stack([r["out"] for r in res.results], axis=0)
    if res.exec_time_ns is not None:
        print(f"HW exec time: {res.exec_time_ns} ns")
    kernel.last_result = res
    return out
